# revision 16
# baseline (speedup 1.0000x reference)
"""GQA attention kernel for Trainium2, 8 NeuronCores.

Sharding: 8 cores = 2 (batch) x 4 (kv-head groups; 4 q heads each).
Per core (batch b, group g), bf16 compute with exact ACT exp:
  phase 1: qT/kT/vT projections (combined [Wk|Wv] stationary, stationary
           reuse across a q-slice pair), RoPE via swap-permutation matmul
           + elementwise tables (softmax scale folded into tables),
           V transposed to natural layout with ones cols for sumexp.
  phase 2: per (head-pair mb, q-slice sl), per key block kb: the two
           heads' scores run as two concurrent row-tiled K=64 matmuls
           (array rows 0-63 / 64-127) into one 2-bank PSUM tile
           [128,1024]; ONE ACT exp evacuates both; AV accumulates into
           ps_oA[65,512] ([V|1] stationary) and ps_oB[128,512]
           ([0..0|1|V] stationary so the odd head lands at partitions
           64-127 and its sumexp at partition 63).
  norm:    sumexp rows staged to SBUF, DMA-scattered onto partitions
           0-15, one batched reciprocal per slice-pair, then one
           broadcast-matmul (F16 selector) + one in-place [128,512]
           multiply per combo.
  phase 3: out_nat[l,:] = attnT.T @ Wo_g (natural layout). Host sums the
           4 partials per batch (row-parallel out_proj all-reduce).
"""
import numpy as np
import ml_dtypes

import concourse.bass as bass
import concourse.mybir as mybir
import concourse.tile as tile
from concourse import bacc
from concourse.bass_utils import run_bass_kernel_spmd

L = 2048
D = 1024
HD = 64
P = 128
NKB = L // 128          # 16 key blocks of 128
F32 = mybir.dt.float32
BF16 = mybir.dt.bfloat16
EXP = mybir.ActivationFunctionType.Exp

_cache = {}


def build_program():
    nc = bacc.Bacc()
    xT_d = nc.dram_tensor("xT", [D, L], BF16, kind="ExternalInput")
    wq_d = nc.dram_tensor("wq", [D, 256], BF16, kind="ExternalInput")
    wkv_d = nc.dram_tensor("wkv", [D, P], BF16, kind="ExternalInput")
    wo_d = nc.dram_tensor("wo", [256, D], BF16, kind="ExternalInput")
    cos_d = nc.dram_tensor("cosT2", [P, L], BF16, kind="ExternalInput")
    sin_d = nc.dram_tensor("sinTS2", [P, L], BF16, kind="ExternalInput")
    s2_d = nc.dram_tensor("S2", [P, P], BF16, kind="ExternalInput")
    eye_d = nc.dram_tensor("EYE", [HD, HD], BF16, kind="ExternalInput")
    f16_d = nc.dram_tensor("F16", [4, 2 * P], BF16, kind="ExternalInput")
    out_d = nc.dram_tensor("out_nat", [L, D], F32, kind="ExternalOutput")

    with tile.TileContext(nc) as tc:
        with (
            tc.tile_pool(name="const", bufs=1) as const,
            tc.tile_pool(name="xc", bufs=8) as xcp,
            tc.tile_pool(name="work", bufs=3) as work,
            tc.tile_pool(name="probs", bufs=3) as probs_p,
            tc.tile_pool(name="outsb", bufs=3) as outsb_p,
            tc.tile_pool(name="psBig", bufs=2, space="PSUM") as psBig,
            tc.tile_pool(name="psO", bufs=1, space="PSUM") as psO,
            tc.tile_pool(name="psX", bufs=2, space="PSUM") as psX,
        ):
            # ---- constants ----
            wq_sb = const.tile([P, 8, 256], BF16)
            nc.sync.dma_start(out=wq_sb, in_=wq_d.rearrange("(c p) n -> p c n", p=P))
            wkv_sb = const.tile([P, 8, P], BF16)
            nc.sync.dma_start(out=wkv_sb, in_=wkv_d.rearrange("(c p) n -> p c n", p=P))
            wo_sb = const.tile([P, 2, D], BF16)
            nc.sync.dma_start(out=wo_sb, in_=wo_d.rearrange("(c p) n -> p c n", p=P))
            cos_sb = const.tile([P, L], BF16)
            nc.sync.dma_start(out=cos_sb, in_=cos_d[:, :])
            sin_sb = const.tile([P, L], BF16)
            nc.sync.dma_start(out=sin_sb, in_=sin_d[:, :])
            s2_sb = const.tile([P, P], BF16)
            nc.sync.dma_start(out=s2_sb, in_=s2_d[:, :])
            eye_sb = const.tile([HD, HD], BF16)
            nc.sync.dma_start(out=eye_sb, in_=eye_d[:, :])
            f16_sb = const.tile([4, 2, P], BF16)
            nc.sync.dma_start(out=f16_sb, in_=f16_d.rearrange("p (c n) -> p c n", n=P))

            qTr = const.tile([P, 2, L], BF16)     # rope'd qT, 2 m-blocks
            kT2 = const.tile([P, L], BF16)        # rope'd kT, both halves
            vToc = const.tile([HD, L], BF16)      # vT staging
            # cols: 0-31 zero, 32 ones, 33-63 zero, 64-127 V, 128 ones
            vfull = const.tile([P, NKB, 130], BF16)
            attnT = const.tile([P, 2, L], BF16)
            stg = const.tile([65, 8, 512], F32)   # sumexp staging rows 32/64
            se_sb = const.tile([4, 4, 512], F32)   # sumexp rows, [:, sl, :]
            rec_sb = const.tile([4, 4, 512], BF16)  # 1/sumexp
            nc.vector.memset(vfull[:, :, 0:64], 0.0)
            nc.vector.memset(vfull[:, :, 32], 1.0)
            nc.vector.memset(vfull[:, :, 128], 1.0)

            # ---- phase 1: projections + rope ----
            def rope_k(kvt, s2i, qs):
                sl_lo = slice(s2i * 512, (s2i + 1) * 512)
                nc.vector.tensor_copy(vToc[:, qs:qs + 512], kvt[HD:P, sl_lo])
                t2 = work.tile([HD, 512], BF16, tag="ks")
                nc.vector.tensor_mul(t2, kvt[:HD, sl_lo], sin_sb[:HD, qs:qs + 512])
                ps_ks = psX.tile([HD, 512], F32, tag="scratch")
                nc.tensor.matmul(ps_ks, s2_sb[:HD, :HD], t2)
                t1 = work.tile([HD, 512], BF16, tag="kc")
                nc.vector.tensor_mul(t1, kvt[:HD, sl_lo], cos_sb[:HD, qs:qs + 512])
                nc.vector.tensor_add(kT2[:HD, qs:qs + 512], t1, ps_ks)

            def rope_q(qt, mb, s2i, qs):
                sl_lo = slice(s2i * 512, (s2i + 1) * 512)
                u2 = work.tile([P, 512], BF16, tag="qs")
                nc.vector.tensor_mul(u2, qt[:, sl_lo], sin_sb[:, qs:qs + 512])
                ps_qs = psX.tile([P, 512], F32, tag="scratch")
                nc.tensor.matmul(ps_qs, s2_sb, u2)
                u1 = work.tile([P, 512], BF16, tag="qc")
                nc.vector.tensor_mul(u1, qt[:, sl_lo], cos_sb[:, qs:qs + 512])
                nc.vector.tensor_add(qTr[:, mb, qs:qs + 512], u1, ps_qs)

            def kv_phase(slp):
                qs2 = slp * 1024
                kvt = psBig.tile([P, 1024], F32, tag="spair")
                for dc in range(8):
                    xc = xcp.tile([P, 1024], BF16, tag="xc")
                    nc.gpsimd.dma_start(
                        out=xc, in_=xT_d[dc * P:(dc + 1) * P, qs2:qs2 + 1024])
                    st, sp = (dc == 0), (dc == 7)
                    for s2i in range(2):
                        xs = slice(s2i * 512, (s2i + 1) * 512)
                        nc.tensor.matmul(kvt[:, xs], wkv_sb[:, dc, :], xc[:, xs],
                                         start=st, stop=sp, skip_group_check=True)
                for s2i in range(2):
                    rope_k(kvt, s2i, qs2 + s2i * 512)
                # duplicate kT into the upper partition half (row-tile B operand)
                nc.sync.dma_start(out=kT2[HD:P, qs2:qs2 + 1024],
                                  in_=kT2[:HD, qs2:qs2 + 1024])
                # V transpose to natural layout, written into vfull cols 64-127
                for kb in range(slp * 8, slp * 8 + 8):
                    ps_vt = psX.tile([P, HD], BF16, tag="scratch")
                    nc.tensor.transpose(ps_vt, vToc[:, kb * P:(kb + 1) * P], eye_sb)
                    nc.vector.tensor_copy(vfull[:, kb, 64:128], ps_vt)

            def qproj(sl):
                qs = sl * 512
                q0t = psX.tile([P, 512], F32, tag="scratch")
                q1t = psX.tile([P, 512], F32, tag="scratch")
                for dc in range(8):
                    xc = xcp.tile([P, 512], BF16, tag="xq")
                    nc.gpsimd.dma_start(
                        out=xc, in_=xT_d[dc * P:(dc + 1) * P, qs:qs + 512])
                    st, sp = (dc == 0), (dc == 7)
                    nc.tensor.matmul(q0t, wq_sb[:, dc, 0:P], xc,
                                     start=st, stop=sp, skip_group_check=True)
                    nc.tensor.matmul(q1t, wq_sb[:, dc, P:256], xc,
                                     start=st, stop=sp, skip_group_check=True)
                rope_q(q0t, 0, 0, qs)
                rope_q(q1t, 1, 0, qs)

            # ---- phase 2: attention ----
            def attention(sl, mb):
                qs = sl * 512
                cs = sl * 2 + mb                # stage slot
                ps_oA = psO.tile([HD + 1, 512], F32, tag="oA")
                ps_oB = psO.tile([P, 512], F32, tag="oB")
                for kb in range(NKB):
                    kc = slice(kb * P, (kb + 1) * P)
                    ps_s = psBig.tile([P, 1024], F32, tag="spair")
                    nc.tensor.matmul(
                        ps_s[:, 0:512], kT2[0:HD, kc],
                        qTr[0:HD, mb, qs:qs + 512], tile_position=(0, 0))
                    nc.tensor.matmul(
                        ps_s[:, 512:1024], kT2[HD:P, kc],
                        qTr[HD:P, mb, qs:qs + 512], tile_position=(HD, 0))
                    pt = probs_p.tile([P, 1024], BF16, tag="probs")
                    nc.scalar.activation(pt, ps_s, EXP)
                    st, sp = (kb == 0), (kb == NKB - 1)
                    nc.tensor.matmul(ps_oA, vfull[:, kb, 64:129], pt[:, 0:512],
                                     start=st, stop=sp, skip_group_check=True)
                    nc.tensor.matmul(ps_oB, vfull[:, kb, 0:128], pt[:, 512:1024],
                                     start=st, stop=sp, skip_group_check=True)
                # unnormalized attn + staged sumexp rows (partitions 64 / 32)
                nc.vector.tensor_copy(attnT[0:HD, mb, qs:qs + 512], ps_oA[0:HD, :])
                nc.vector.tensor_copy(attnT[HD:P, mb, qs:qs + 512], ps_oB[HD:P, :])
                nc.vector.tensor_copy(stg[64:65, cs % 8, :], ps_oA[HD:HD + 1, :])
                nc.vector.tensor_copy(stg[32:33, cs % 8, :], ps_oB[32:33, :])
                nc.sync.dma_start(
                    out=se_sb[2 * mb:2 * mb + 1, sl, :], in_=stg[64:65, cs % 8, :])
                nc.sync.dma_start(
                    out=se_sb[2 * mb + 1:2 * mb + 2, sl, :], in_=stg[32:33, cs % 8, :])

            def norm(sl):
                with nc.allow_low_precision(reason="1/sumexp consumed in bf16"):
                    nc.vector.reciprocal(rec_sb[:, sl, :], se_sb[:, sl, :])
                qs = sl * 512
                for mb in range(2):
                    ps_b = psX.tile([P, 512], F32, tag="scratch")
                    nc.tensor.matmul(ps_b, f16_sb[:, mb, :], rec_sb[:, sl, :])
                    dst = attnT[:, mb, qs:qs + 512]
                    nc.vector.tensor_mul(dst, dst, ps_b)

            def outproj(sl):
                for lb in range(sl * 4, sl * 4 + 4):
                    lc = slice(lb * P, (lb + 1) * P)
                    psn0 = psX.tile([P, 512], F32, tag="scratch")
                    psn1 = psX.tile([P, 512], F32, tag="scratch")
                    psn = [psn0, psn1]
                    for ab in range(2):
                        st, sp = (ab == 0), (ab == 1)
                        for nh in range(2):
                            nc.tensor.matmul(
                                psn[nh], attnT[:, ab, lc],
                                wo_sb[:, ab, nh * 512:(nh + 1) * 512],
                                start=st, stop=sp, skip_group_check=True)
                    for nh in range(2):
                        osb = outsb_p.tile([P, 512], F32, tag="osb")
                        nc.vector.tensor_copy(osb, psn[nh])
                        nc.sync.dma_start(
                            out=out_d[lc, nh * 512:(nh + 1) * 512], in_=osb)

            kv_phase(0)
            qproj(0)
            kv_phase(1)
            qproj(1)
            attention(0, 0)
            attention(0, 1)
            norm(0)
            attention(1, 0)
            qproj(2)
            attention(1, 1)
            norm(1)
            outproj(0)
            attention(2, 0)
            qproj(3)
            attention(2, 1)
            norm(2)
            outproj(1)
            attention(3, 0)
            attention(3, 1)
            norm(3)
            outproj(2)
            outproj(3)

    nc.compile()
    return nc


BF = ml_dtypes.bfloat16


def _host_tables():
    inv_freq = 1.0 / (10000.0 ** (np.arange(0, HD, 2, dtype=np.float32) / HD))
    t = np.arange(L, dtype=np.float32)
    freqs = t[:, None] * inv_freq[None, :]
    emb = np.concatenate([freqs, freqs], axis=-1)
    s8 = np.float32(8.0 ** -0.5)
    cosT = np.cos(emb).T.astype(np.float32)
    sinT = np.sin(emb).T.astype(np.float32)
    # sign fold for multiply-then-swap rope order
    sinTS = np.concatenate([sinT[:32], -sinT[32:]], axis=0)
    cosT2 = np.ascontiguousarray(np.concatenate([cosT, cosT], axis=0) * s8).astype(BF)
    sinTS2 = np.ascontiguousarray(np.concatenate([sinTS, sinTS], axis=0) * s8).astype(BF)
    S = np.zeros((64, 64), np.float32)
    for j in range(64):
        S[(j + 32) % 64, j] = 1.0
    S2 = np.zeros((128, 128), np.float32)
    S2[:64, :64] = S
    S2[64:, 64:] = S
    S2 = S2.astype(BF)
    eye = np.eye(HD, dtype=np.float32).astype(BF)
    # F16[:, mb, :]: broadcast-selector — out rows 0-63 get rec row 2mb,
    # rows 64-127 get rec row 2mb+1
    F16 = np.zeros((4, 2, P), np.float32)
    for mb in range(2):
        F16[2 * mb, mb, 0:HD] = 1.0
        F16[2 * mb + 1, mb, HD:P] = 1.0
    F16 = np.ascontiguousarray(F16.reshape(4, 2 * P)).astype(BF)
    return cosT2, sinTS2, S2, eye, F16


def kernel(x, Wq, Wk, Wv, Wo, _trace=False, _tmpdir=None):
    x = np.asarray(x, np.float32)
    Wq = np.asarray(Wq, np.float32)
    Wk = np.asarray(Wk, np.float32)
    Wv = np.asarray(Wv, np.float32)
    Wo = np.asarray(Wo, np.float32)
    B = x.shape[0]
    cosT2, sinTS2, S2, eye, F16 = _host_tables()

    if "nc" not in _cache:
        _cache["nc"] = build_program()
    nc = _cache["nc"]

    xT_b = [np.ascontiguousarray(x[b].T).astype(BF) for b in range(B)]
    wq_g, wkv_g, wo_g = [], [], []
    for g in range(4):
        wq_g.append(np.ascontiguousarray(Wq[:, g * 256:(g + 1) * 256]).astype(BF))
        wkv_g.append(np.ascontiguousarray(np.concatenate(
            [Wk[:, g * HD:(g + 1) * HD], Wv[:, g * HD:(g + 1) * HD]],
            axis=1)).astype(BF))
        wo_g.append(np.ascontiguousarray(Wo[g * 256:(g + 1) * 256, :]).astype(BF))

    in_maps = []
    for c in range(8):
        b, g = c // 4, c % 4
        in_maps.append({
            "xT": xT_b[b], "wq": wq_g[g], "wkv": wkv_g[g], "wo": wo_g[g],
            "cosT2": cosT2, "sinTS2": sinTS2, "S2": S2, "EYE": eye, "F16": F16,
        })

    res = run_bass_kernel_spmd(
        nc, in_maps, list(range(8)), trace=_trace, tmpdir=_tmpdir)
    out = np.zeros((B, L, D), np.float32)
    for c in range(8):
        b = c // 4
        out[b] += res.results[c]["out_nat"]
    if _trace:
        kernel.last_exec_time_ns = res.exec_time_ns
        kernel.last_results = res
    return out


# revision 17
# speedup vs baseline: 1.0701x; 1.0701x over previous
"""GQA attention kernel for Trainium2, 8 NeuronCores.

Sharding: 8 cores = 2 (batch) x 4 (kv-head groups; 4 q heads each).
Per core (batch b, group g), bf16 compute with exact ACT exp:
  phase 1: qT/kT/vT projections (combined [Wk|Wv] stationary, stationary
           reuse across a q-slice pair), RoPE via swap-permutation matmul
           + elementwise tables (softmax scale folded into tables),
           V transposed to natural layout with ones cols for sumexp.
  phase 2: per (head-pair mb, q-slice sl), per key block kb: the two
           heads' scores run as two concurrent row-tiled K=64 matmuls
           (array rows 0-63 / 64-127) into one 2-bank PSUM tile
           [128,1024]; ONE ACT exp evacuates both; AV accumulates into
           ps_oA[65,512] ([V|1] stationary) and ps_oB[128,512]
           ([0..0|1|V] stationary so the odd head lands at partitions
           64-127 and its sumexp at partition 63).
  norm:    sumexp rows staged to SBUF, DMA-scattered onto partitions
           0-15, one batched reciprocal per slice-pair, then one
           broadcast-matmul (F16 selector) + one in-place [128,512]
           multiply per combo.
  phase 3: out_nat[l,:] = attnT.T @ Wo_g (natural layout). Host sums the
           4 partials per batch (row-parallel out_proj all-reduce).
"""
import numpy as np
import ml_dtypes

import concourse.bass as bass
import concourse.mybir as mybir
import concourse.tile as tile
from concourse import bacc
from concourse.bass_utils import run_bass_kernel_spmd

L = 2048
D = 1024
HD = 64
P = 128
NKB = L // 128          # 16 key blocks of 128
F32 = mybir.dt.float32
BF16 = mybir.dt.bfloat16
EXP = mybir.ActivationFunctionType.Exp

_cache = {}


def build_program():
    nc = bacc.Bacc()
    xT_d = nc.dram_tensor("xT", [D, L], BF16, kind="ExternalInput")
    wq_d = nc.dram_tensor("wq", [D, 256], BF16, kind="ExternalInput")
    wkv_d = nc.dram_tensor("wkv", [D, P], BF16, kind="ExternalInput")
    wo_d = nc.dram_tensor("wo", [256, D], BF16, kind="ExternalInput")
    cos_d = nc.dram_tensor("cosT2", [P, L], BF16, kind="ExternalInput")
    sin_d = nc.dram_tensor("sinTS2", [P, L], BF16, kind="ExternalInput")
    s2_d = nc.dram_tensor("S2", [P, P], BF16, kind="ExternalInput")
    eye_d = nc.dram_tensor("EYE", [HD, HD], BF16, kind="ExternalInput")
    f16_d = nc.dram_tensor("F16", [4, 2 * P], BF16, kind="ExternalInput")
    out_d = nc.dram_tensor("out_nat", [L, D], F32, kind="ExternalOutput")

    with tile.TileContext(nc) as tc:
        with (
            tc.tile_pool(name="const", bufs=1) as const,
            tc.tile_pool(name="xc", bufs=8) as xcp,
            tc.tile_pool(name="work", bufs=3) as work,
            tc.tile_pool(name="probs", bufs=3) as probs_p,
            tc.tile_pool(name="outsb", bufs=3) as outsb_p,
            tc.tile_pool(name="psBig", bufs=2, space="PSUM") as psBig,
            tc.tile_pool(name="psO", bufs=1, space="PSUM") as psO,
            tc.tile_pool(name="psX", bufs=2, space="PSUM") as psX,
        ):
            # ---- constants ----
            wq_sb = const.tile([P, 8, 256], BF16)
            nc.sync.dma_start(out=wq_sb, in_=wq_d.rearrange("(c p) n -> p c n", p=P))
            wkv_sb = const.tile([P, 8, P], BF16)
            nc.sync.dma_start(out=wkv_sb, in_=wkv_d.rearrange("(c p) n -> p c n", p=P))
            wo_sb = const.tile([P, 2, D], BF16)
            nc.sync.dma_start(out=wo_sb, in_=wo_d.rearrange("(c p) n -> p c n", p=P))
            cos_sb = const.tile([P, L], BF16)
            nc.sync.dma_start(out=cos_sb, in_=cos_d[:, :])
            sin_sb = const.tile([P, L], BF16)
            nc.sync.dma_start(out=sin_sb, in_=sin_d[:, :])
            s2_sb = const.tile([P, P], BF16)
            nc.sync.dma_start(out=s2_sb, in_=s2_d[:, :])
            eye_sb = const.tile([HD, HD], BF16)
            nc.sync.dma_start(out=eye_sb, in_=eye_d[:, :])
            f16_sb = const.tile([4, 2, P], BF16)
            nc.sync.dma_start(out=f16_sb, in_=f16_d.rearrange("p (c n) -> p c n", n=P))

            qTr = const.tile([P, 2, L], BF16)     # rope'd qT, 2 m-blocks
            kT2 = const.tile([P, L], BF16)        # rope'd kT, both halves
            vToc = const.tile([HD, L], BF16)      # vT staging
            # cols: 0-31 zero, 32 ones, 33-63 zero, 64-127 V, 128 ones
            vfull = const.tile([P, NKB, 130], BF16)
            attnT = const.tile([P, 2, L], BF16)
            stg = const.tile([65, 8, 512], F32)   # sumexp staging rows 32/64
            se_sb = const.tile([4, 4, 512], F32)   # sumexp rows, [:, sl, :]
            rec_sb = const.tile([4, 4, 512], BF16)  # 1/sumexp
            nc.vector.memset(vfull[:, :, 0:64], 0.0)
            nc.vector.memset(vfull[:, :, 32], 1.0)
            nc.vector.memset(vfull[:, :, 128], 1.0)

            # ---- phase 1: projections + rope ----
            def rope_k(kvt, s2i, qs):
                sl_lo = slice(s2i * 512, (s2i + 1) * 512)
                nc.vector.tensor_copy(vToc[:, qs:qs + 512], kvt[HD:P, sl_lo])
                t2 = work.tile([HD, 512], BF16, tag="ks")
                nc.vector.tensor_mul(t2, kvt[:HD, sl_lo], sin_sb[:HD, qs:qs + 512])
                ps_ks = psX.tile([HD, 512], F32, tag="scratch")
                nc.tensor.matmul(ps_ks, s2_sb[:HD, :HD], t2)
                t1 = work.tile([HD, 512], BF16, tag="kc")
                nc.vector.tensor_mul(t1, kvt[:HD, sl_lo], cos_sb[:HD, qs:qs + 512])
                nc.vector.tensor_add(kT2[:HD, qs:qs + 512], t1, ps_ks)

            def rope_q(qt, mb, s2i, qs):
                sl_lo = slice(s2i * 512, (s2i + 1) * 512)
                u2 = work.tile([P, 512], BF16, tag="qs")
                nc.vector.tensor_mul(u2, qt[:, sl_lo], sin_sb[:, qs:qs + 512])
                ps_qs = psX.tile([P, 512], F32, tag="scratch")
                nc.tensor.matmul(ps_qs, s2_sb, u2)
                u1 = work.tile([P, 512], BF16, tag="qc")
                nc.vector.tensor_mul(u1, qt[:, sl_lo], cos_sb[:, qs:qs + 512])
                nc.vector.tensor_add(qTr[:, mb, qs:qs + 512], u1, ps_qs)

            def kv_phase(slp):
                qs2 = slp * 1024
                kvt = psBig.tile([P, 1024], F32, tag="spair")
                for dc in range(8):
                    xc = xcp.tile([P, 1024], BF16, tag="xc")
                    nc.gpsimd.dma_start(
                        out=xc, in_=xT_d[dc * P:(dc + 1) * P, qs2:qs2 + 1024])
                    st, sp = (dc == 0), (dc == 7)
                    for s2i in range(2):
                        xs = slice(s2i * 512, (s2i + 1) * 512)
                        nc.tensor.matmul(kvt[:, xs], wkv_sb[:, dc, :], xc[:, xs],
                                         start=st, stop=sp, skip_group_check=True)
                for s2i in range(2):
                    rope_k(kvt, s2i, qs2 + s2i * 512)
                # duplicate kT into the upper partition half (row-tile B operand)
                nc.sync.dma_start(out=kT2[HD:P, qs2:qs2 + 1024],
                                  in_=kT2[:HD, qs2:qs2 + 1024])
                # V transpose to natural layout, written into vfull cols 64-127
                for kb in range(slp * 8, slp * 8 + 8):
                    ps_vt = psX.tile([P, HD], BF16, tag="scratch")
                    nc.tensor.transpose(ps_vt, vToc[:, kb * P:(kb + 1) * P], eye_sb)
                    nc.vector.tensor_copy(vfull[:, kb, 64:128], ps_vt)

            def qproj(sl):
                qs = sl * 512
                q0t = psX.tile([P, 512], F32, tag="scratch")
                q1t = psX.tile([P, 512], F32, tag="scratch")
                for dc in range(8):
                    xc = xcp.tile([P, 512], BF16, tag="xq")
                    nc.gpsimd.dma_start(
                        out=xc, in_=xT_d[dc * P:(dc + 1) * P, qs:qs + 512])
                    st, sp = (dc == 0), (dc == 7)
                    nc.tensor.matmul(q0t, wq_sb[:, dc, 0:P], xc,
                                     start=st, stop=sp, skip_group_check=True)
                    nc.tensor.matmul(q1t, wq_sb[:, dc, P:256], xc,
                                     start=st, stop=sp, skip_group_check=True)
                rope_q(q0t, 0, 0, qs)
                rope_q(q1t, 1, 0, qs)

            # ---- phase 2: attention ----
            av_state = {}

            def attention(sl, mb, kb_lo=0, kb_hi=NKB):
                qs = sl * 512
                cs = sl * 2 + mb                # stage slot
                if kb_lo == 0:
                    ps_oA = psO.tile([HD + 1, 512], F32, tag="oA")
                    ps_oB = psO.tile([P, 512], F32, tag="oB")
                    av_state[(sl, mb)] = (ps_oA, ps_oB)
                else:
                    ps_oA, ps_oB = av_state[(sl, mb)]
                for kb in range(kb_lo, kb_hi):
                    kc = slice(kb * P, (kb + 1) * P)
                    ps_s = psBig.tile([P, 1024], F32, tag="spair")
                    nc.tensor.matmul(
                        ps_s[:, 0:512], kT2[0:HD, kc],
                        qTr[0:HD, mb, qs:qs + 512], tile_position=(0, 0))
                    nc.tensor.matmul(
                        ps_s[:, 512:1024], kT2[HD:P, kc],
                        qTr[HD:P, mb, qs:qs + 512], tile_position=(HD, 0))
                    pt = probs_p.tile([P, 1024], BF16, tag="probs")
                    nc.scalar.activation(pt, ps_s, EXP)
                    st, sp = (kb == 0), (kb == NKB - 1)
                    nc.tensor.matmul(ps_oA, vfull[:, kb, 64:129], pt[:, 0:512],
                                     start=st, stop=sp, skip_group_check=True)
                    nc.tensor.matmul(ps_oB, vfull[:, kb, 0:128], pt[:, 512:1024],
                                     start=st, stop=sp, skip_group_check=True)
                if kb_hi < NKB:
                    return
                # unnormalized attn + staged sumexp rows (partitions 64 / 32)
                nc.vector.tensor_copy(attnT[0:HD, mb, qs:qs + 512], ps_oA[0:HD, :])
                nc.vector.tensor_copy(attnT[HD:P, mb, qs:qs + 512], ps_oB[HD:P, :])
                nc.vector.tensor_copy(stg[64:65, cs % 8, :], ps_oA[HD:HD + 1, :])
                nc.vector.tensor_copy(stg[32:33, cs % 8, :], ps_oB[32:33, :])
                nc.sync.dma_start(
                    out=se_sb[2 * mb:2 * mb + 1, sl, :], in_=stg[64:65, cs % 8, :])
                nc.sync.dma_start(
                    out=se_sb[2 * mb + 1:2 * mb + 2, sl, :], in_=stg[32:33, cs % 8, :])

            def norm(sl):
                with nc.allow_low_precision(reason="1/sumexp consumed in bf16"):
                    nc.vector.reciprocal(rec_sb[:, sl, :], se_sb[:, sl, :])
                qs = sl * 512
                for mb in range(2):
                    ps_b = psX.tile([P, 512], F32, tag="scratch")
                    nc.tensor.matmul(ps_b, f16_sb[:, mb, :], rec_sb[:, sl, :])
                    dst = attnT[:, mb, qs:qs + 512]
                    nc.vector.tensor_mul(dst, dst, ps_b)

            def outproj(sl):
                for lb in range(sl * 4, sl * 4 + 4):
                    lc = slice(lb * P, (lb + 1) * P)
                    psn0 = psX.tile([P, 512], F32, tag="scratch")
                    psn1 = psX.tile([P, 512], F32, tag="scratch")
                    psn = [psn0, psn1]
                    for ab in range(2):
                        st, sp = (ab == 0), (ab == 1)
                        for nh in range(2):
                            nc.tensor.matmul(
                                psn[nh], attnT[:, ab, lc],
                                wo_sb[:, ab, nh * 512:(nh + 1) * 512],
                                start=st, stop=sp, skip_group_check=True)
                    for nh in range(2):
                        osb = outsb_p.tile([P, 512], F32, tag="osb")
                        nc.vector.tensor_copy(osb, psn[nh])
                        nc.sync.dma_start(
                            out=out_d[lc, nh * 512:(nh + 1) * 512], in_=osb)

            kv_phase(0)
            qproj(0)
            attention(0, 0, 0, 8)
            kv_phase(1)
            qproj(1)
            attention(0, 0, 8, NKB)
            attention(0, 1)
            attention(1, 0)
            qproj(2)
            attention(1, 1)
            attention(2, 0)
            qproj(3)
            attention(2, 1)
            attention(3, 0)
            attention(3, 1)
            norm(0)
            outproj(0)
            norm(1)
            outproj(1)
            norm(2)
            outproj(2)
            norm(3)
            outproj(3)

    nc.compile()
    return nc


BF = ml_dtypes.bfloat16


def _host_tables():
    inv_freq = 1.0 / (10000.0 ** (np.arange(0, HD, 2, dtype=np.float32) / HD))
    t = np.arange(L, dtype=np.float32)
    freqs = t[:, None] * inv_freq[None, :]
    emb = np.concatenate([freqs, freqs], axis=-1)
    s8 = np.float32(8.0 ** -0.5)
    cosT = np.cos(emb).T.astype(np.float32)
    sinT = np.sin(emb).T.astype(np.float32)
    # sign fold for multiply-then-swap rope order
    sinTS = np.concatenate([sinT[:32], -sinT[32:]], axis=0)
    cosT2 = np.ascontiguousarray(np.concatenate([cosT, cosT], axis=0) * s8).astype(BF)
    sinTS2 = np.ascontiguousarray(np.concatenate([sinTS, sinTS], axis=0) * s8).astype(BF)
    S = np.zeros((64, 64), np.float32)
    for j in range(64):
        S[(j + 32) % 64, j] = 1.0
    S2 = np.zeros((128, 128), np.float32)
    S2[:64, :64] = S
    S2[64:, 64:] = S
    S2 = S2.astype(BF)
    eye = np.eye(HD, dtype=np.float32).astype(BF)
    # F16[:, mb, :]: broadcast-selector — out rows 0-63 get rec row 2mb,
    # rows 64-127 get rec row 2mb+1
    F16 = np.zeros((4, 2, P), np.float32)
    for mb in range(2):
        F16[2 * mb, mb, 0:HD] = 1.0
        F16[2 * mb + 1, mb, HD:P] = 1.0
    F16 = np.ascontiguousarray(F16.reshape(4, 2 * P)).astype(BF)
    return cosT2, sinTS2, S2, eye, F16


def kernel(x, Wq, Wk, Wv, Wo, _trace=False, _tmpdir=None):
    x = np.asarray(x, np.float32)
    Wq = np.asarray(Wq, np.float32)
    Wk = np.asarray(Wk, np.float32)
    Wv = np.asarray(Wv, np.float32)
    Wo = np.asarray(Wo, np.float32)
    B = x.shape[0]
    cosT2, sinTS2, S2, eye, F16 = _host_tables()

    if "nc" not in _cache:
        _cache["nc"] = build_program()
    nc = _cache["nc"]

    xT_b = [np.ascontiguousarray(x[b].T).astype(BF) for b in range(B)]
    wq_g, wkv_g, wo_g = [], [], []
    for g in range(4):
        wq_g.append(np.ascontiguousarray(Wq[:, g * 256:(g + 1) * 256]).astype(BF))
        wkv_g.append(np.ascontiguousarray(np.concatenate(
            [Wk[:, g * HD:(g + 1) * HD], Wv[:, g * HD:(g + 1) * HD]],
            axis=1)).astype(BF))
        wo_g.append(np.ascontiguousarray(Wo[g * 256:(g + 1) * 256, :]).astype(BF))

    in_maps = []
    for c in range(8):
        b, g = c // 4, c % 4
        in_maps.append({
            "xT": xT_b[b], "wq": wq_g[g], "wkv": wkv_g[g], "wo": wo_g[g],
            "cosT2": cosT2, "sinTS2": sinTS2, "S2": S2, "EYE": eye, "F16": F16,
        })

    res = run_bass_kernel_spmd(
        nc, in_maps, list(range(8)), trace=_trace, tmpdir=_tmpdir)
    out = np.zeros((B, L, D), np.float32)
    for c in range(8):
        b = c // 4
        out[b] += res.results[c]["out_nat"]
    if _trace:
        kernel.last_exec_time_ns = res.exec_time_ns
        kernel.last_results = res
    return out


# revision 19
# speedup vs baseline: 1.1205x; 1.0472x over previous
"""GQA attention kernel for Trainium2, 8 NeuronCores.

Sharding: 8 cores = 2 (batch) x 4 (kv-head groups; 4 q heads each).
Per core (batch b, group g), bf16 compute with exact ACT exp:
  phase 1: qT/kT/vT projections (combined [Wk|Wv] stationary, stationary
           reuse across a q-slice pair), RoPE via swap-permutation matmul
           + elementwise tables (softmax scale folded into tables),
           V transposed to natural layout with ones cols for sumexp.
  phase 2: per (head-pair mb, q-slice sl), per key block kb: the two
           heads' scores run as two concurrent row-tiled K=64 matmuls
           (array rows 0-63 / 64-127) into one 2-bank PSUM tile
           [128,1024]; ONE ACT exp evacuates both; AV accumulates into
           ps_oA[65,512] ([V|1] stationary) and ps_oB[128,512]
           ([0..0|1|V] stationary so the odd head lands at partitions
           64-127 and its sumexp at partition 63).
  norm:    sumexp rows staged to SBUF, DMA-scattered onto partitions
           0-15, one batched reciprocal per slice-pair, then one
           broadcast-matmul (F16 selector) + one in-place [128,512]
           multiply per combo.
  phase 3: out_nat[l,:] = attnT.T @ Wo_g (natural layout). Host sums the
           4 partials per batch (row-parallel out_proj all-reduce).
"""
import numpy as np
import ml_dtypes

import concourse.bass as bass
import concourse.mybir as mybir
import concourse.tile as tile
from concourse import bacc
from concourse.bass_utils import run_bass_kernel_spmd

L = 2048
D = 1024
HD = 64
P = 128
NKB = L // 128          # 16 key blocks of 128
F32 = mybir.dt.float32
BF16 = mybir.dt.bfloat16
EXP = mybir.ActivationFunctionType.Exp

_cache = {}


def build_program():
    nc = bacc.Bacc()
    xT_d = nc.dram_tensor("xT", [D, L], BF16, kind="ExternalInput")
    wq_d = nc.dram_tensor("wq", [D, 256], BF16, kind="ExternalInput")
    wkv_d = nc.dram_tensor("wkv", [D, P], BF16, kind="ExternalInput")
    wo_d = nc.dram_tensor("wo", [256, D], BF16, kind="ExternalInput")
    cos_d = nc.dram_tensor("cosT2", [P, L], BF16, kind="ExternalInput")
    sin_d = nc.dram_tensor("sinTS2", [P, L], BF16, kind="ExternalInput")
    s2_d = nc.dram_tensor("S2", [P, P], BF16, kind="ExternalInput")
    eye_d = nc.dram_tensor("EYE", [HD, HD], BF16, kind="ExternalInput")
    f16_d = nc.dram_tensor("F16", [4, 2 * P], BF16, kind="ExternalInput")
    out_d = nc.dram_tensor("out_nat", [L, D], F32, kind="ExternalOutput")

    with tile.TileContext(nc) as tc:
        with (
            tc.tile_pool(name="const", bufs=1) as const,
            tc.tile_pool(name="xc", bufs=8) as xcp,
            tc.tile_pool(name="work", bufs=3) as work,
            tc.tile_pool(name="probs", bufs=3) as probs_p,
            tc.tile_pool(name="outsb", bufs=3) as outsb_p,
            tc.tile_pool(name="psBig", bufs=2, space="PSUM") as psBig,
            tc.tile_pool(name="psO", bufs=1, space="PSUM") as psO,
            tc.tile_pool(name="psX", bufs=2, space="PSUM") as psX,
        ):
            # ---- constants (ordered by first use; spread across queues) ----
            wkv_sb = const.tile([P, 8, P], BF16)
            nc.scalar.dma_start(out=wkv_sb, in_=wkv_d.rearrange("(c p) n -> p c n", p=P))
            wq_sb = const.tile([P, 8, 256], BF16)
            nc.scalar.dma_start(out=wq_sb, in_=wq_d.rearrange("(c p) n -> p c n", p=P))
            cos_sb = const.tile([P, L], BF16)
            nc.scalar.dma_start(out=cos_sb, in_=cos_d[:, :])
            sin_sb = const.tile([P, L], BF16)
            nc.scalar.dma_start(out=sin_sb, in_=sin_d[:, :])
            s2_sb = const.tile([P, P], BF16)
            nc.sync.dma_start(out=s2_sb, in_=s2_d[:, :])
            eye_sb = const.tile([HD, HD], BF16)
            nc.sync.dma_start(out=eye_sb, in_=eye_d[:, :])
            wo_sb = const.tile([P, 2, D], BF16)
            nc.sync.dma_start(out=wo_sb, in_=wo_d.rearrange("(c p) n -> p c n", p=P))
            f16_sb = const.tile([4, 2, P], BF16)
            nc.sync.dma_start(out=f16_sb, in_=f16_d.rearrange("p (c n) -> p c n", n=P))

            qTr = const.tile([P, 2, L], BF16)     # rope'd qT, 2 m-blocks
            kT2 = const.tile([P, L], BF16)        # rope'd kT, both halves
            vToc = const.tile([HD, L], BF16)      # vT staging
            # cols: 0-31 zero, 32 ones, 33-63 zero, 64-127 V, 128 ones
            vfull = const.tile([P, NKB, 130], BF16)
            attnT = const.tile([P, 2, L], BF16)
            stg = const.tile([65, 8, 512], F32)   # sumexp staging rows 32/64
            se_sb = const.tile([4, 4, 512], F32)   # sumexp rows, [:, sl, :]
            rec_sb = const.tile([4, 4, 512], BF16)  # 1/sumexp
            nc.vector.memset(vfull[:, :, 0:64], 0.0)
            nc.vector.memset(vfull[:, :, 32], 1.0)
            nc.vector.memset(vfull[:, :, 128], 1.0)

            # ---- phase 1: projections + rope ----
            def rope_k(kvt, s2i, qs):
                sl_lo = slice(s2i * 512, (s2i + 1) * 512)
                nc.vector.tensor_copy(vToc[:, qs:qs + 512], kvt[HD:P, sl_lo])
                t2 = work.tile([HD, 512], BF16, tag="ks")
                nc.vector.tensor_mul(t2, kvt[:HD, sl_lo], sin_sb[:HD, qs:qs + 512])
                ps_ks = psX.tile([HD, 512], F32, tag="scratch")
                nc.tensor.matmul(ps_ks, s2_sb[:HD, :HD], t2)
                t1 = work.tile([HD, 512], BF16, tag="kc")
                nc.vector.tensor_mul(t1, kvt[:HD, sl_lo], cos_sb[:HD, qs:qs + 512])
                nc.vector.tensor_add(kT2[:HD, qs:qs + 512], t1, ps_ks)

            def rope_q(qt, mb, s2i, qs):
                sl_lo = slice(s2i * 512, (s2i + 1) * 512)
                u2 = work.tile([P, 512], BF16, tag="qs")
                nc.vector.tensor_mul(u2, qt[:, sl_lo], sin_sb[:, qs:qs + 512])
                ps_qs = psX.tile([P, 512], F32, tag="scratch")
                nc.tensor.matmul(ps_qs, s2_sb, u2)
                u1 = work.tile([P, 512], BF16, tag="qc")
                nc.vector.tensor_mul(u1, qt[:, sl_lo], cos_sb[:, qs:qs + 512])
                nc.vector.tensor_add(qTr[:, mb, qs:qs + 512], u1, ps_qs)

            def kv_phase(slp):
                qs2 = slp * 1024
                kvt = psBig.tile([P, 1024], F32, tag="spair")
                for dc in range(8):
                    xc = xcp.tile([P, 1024], BF16, tag="xc")
                    nc.gpsimd.dma_start(
                        out=xc, in_=xT_d[dc * P:(dc + 1) * P, qs2:qs2 + 1024])
                    st, sp = (dc == 0), (dc == 7)
                    for s2i in range(2):
                        xs = slice(s2i * 512, (s2i + 1) * 512)
                        nc.tensor.matmul(kvt[:, xs], wkv_sb[:, dc, :], xc[:, xs],
                                         start=st, stop=sp, skip_group_check=True)
                for s2i in range(2):
                    rope_k(kvt, s2i, qs2 + s2i * 512)
                # duplicate kT into the upper partition half (row-tile B operand)
                nc.sync.dma_start(out=kT2[HD:P, qs2:qs2 + 1024],
                                  in_=kT2[:HD, qs2:qs2 + 1024])
                # V transpose to natural layout, written into vfull cols 64-127
                for kb in range(slp * 8, slp * 8 + 8):
                    ps_vt = psX.tile([P, HD], BF16, tag="scratch")
                    nc.tensor.transpose(ps_vt, vToc[:, kb * P:(kb + 1) * P], eye_sb)
                    nc.vector.tensor_copy(vfull[:, kb, 64:128], ps_vt)

            def qproj(sl):
                qs = sl * 512
                q0t = psX.tile([P, 512], F32, tag="scratch")
                q1t = psX.tile([P, 512], F32, tag="scratch")
                for dc in range(8):
                    xc = xcp.tile([P, 512], BF16, tag="xq")
                    nc.scalar.dma_start(
                        out=xc, in_=xT_d[dc * P:(dc + 1) * P, qs:qs + 512])
                    st, sp = (dc == 0), (dc == 7)
                    nc.tensor.matmul(q0t, wq_sb[:, dc, 0:P], xc,
                                     start=st, stop=sp, skip_group_check=True)
                    nc.tensor.matmul(q1t, wq_sb[:, dc, P:256], xc,
                                     start=st, stop=sp, skip_group_check=True)
                rope_q(q0t, 0, 0, qs)
                rope_q(q1t, 1, 0, qs)

            # ---- phase 2: attention ----
            av_state = {}

            def attention(sl, mb, kb_lo=0, kb_hi=NKB):
                qs = sl * 512
                cs = sl * 2 + mb                # stage slot
                if kb_lo == 0:
                    ps_oA = psO.tile([HD + 1, 512], F32, tag="oA")
                    ps_oB = psO.tile([P, 512], F32, tag="oB")
                    av_state[(sl, mb)] = (ps_oA, ps_oB)
                else:
                    ps_oA, ps_oB = av_state[(sl, mb)]
                for kb in range(kb_lo, kb_hi):
                    kc = slice(kb * P, (kb + 1) * P)
                    ps_s = psBig.tile([P, 1024], F32, tag="spair")
                    nc.tensor.matmul(
                        ps_s[:, 0:512], kT2[0:HD, kc],
                        qTr[0:HD, mb, qs:qs + 512], tile_position=(0, 0))
                    nc.tensor.matmul(
                        ps_s[:, 512:1024], kT2[HD:P, kc],
                        qTr[HD:P, mb, qs:qs + 512], tile_position=(HD, 0))
                    pt = probs_p.tile([P, 1024], BF16, tag="probs")
                    nc.scalar.activation(pt, ps_s, EXP)
                    st, sp = (kb == 0), (kb == NKB - 1)
                    nc.tensor.matmul(ps_oA, vfull[:, kb, 64:129], pt[:, 0:512],
                                     start=st, stop=sp, skip_group_check=True)
                    nc.tensor.matmul(ps_oB, vfull[:, kb, 0:128], pt[:, 512:1024],
                                     start=st, stop=sp, skip_group_check=True)
                if kb_hi < NKB:
                    return
                # unnormalized attn + staged sumexp rows (partitions 64 / 32)
                nc.vector.tensor_copy(attnT[0:HD, mb, qs:qs + 512], ps_oA[0:HD, :])
                nc.vector.tensor_copy(attnT[HD:P, mb, qs:qs + 512], ps_oB[HD:P, :])
                nc.vector.tensor_copy(stg[64:65, cs % 8, :], ps_oA[HD:HD + 1, :])
                nc.vector.tensor_copy(stg[32:33, cs % 8, :], ps_oB[32:33, :])
                nc.sync.dma_start(
                    out=se_sb[2 * mb:2 * mb + 1, sl, :], in_=stg[64:65, cs % 8, :])
                nc.sync.dma_start(
                    out=se_sb[2 * mb + 1:2 * mb + 2, sl, :], in_=stg[32:33, cs % 8, :])

            def norm(sl):
                with nc.allow_low_precision(reason="1/sumexp consumed in bf16"):
                    nc.vector.reciprocal(rec_sb[:, sl, :], se_sb[:, sl, :])
                qs = sl * 512
                for mb in range(2):
                    ps_b = psX.tile([P, 512], F32, tag="scratch")
                    nc.tensor.matmul(ps_b, f16_sb[:, mb, :], rec_sb[:, sl, :])
                    dst = attnT[:, mb, qs:qs + 512]
                    nc.vector.tensor_mul(dst, dst, ps_b)

            def outproj(sl):
                for lb in range(sl * 4, sl * 4 + 4):
                    lc = slice(lb * P, (lb + 1) * P)
                    psn0 = psX.tile([P, 512], F32, tag="scratch")
                    psn1 = psX.tile([P, 512], F32, tag="scratch")
                    psn = [psn0, psn1]
                    for ab in range(2):
                        st, sp = (ab == 0), (ab == 1)
                        for nh in range(2):
                            nc.tensor.matmul(
                                psn[nh], attnT[:, ab, lc],
                                wo_sb[:, ab, nh * 512:(nh + 1) * 512],
                                start=st, stop=sp, skip_group_check=True)
                    for nh in range(2):
                        osb = outsb_p.tile([P, 512], F32, tag="osb")
                        nc.vector.tensor_copy(osb, psn[nh])
                        nc.sync.dma_start(
                            out=out_d[lc, nh * 512:(nh + 1) * 512], in_=osb)

            kv_phase(0)
            qproj(0)
            kv_phase(1)
            attention(0, 0)
            qproj(1)
            attention(0, 1)
            attention(1, 0)
            qproj(2)
            attention(1, 1)
            attention(2, 0)
            qproj(3)
            attention(2, 1)
            attention(3, 0)
            attention(3, 1)
            norm(0)
            outproj(0)
            norm(1)
            outproj(1)
            norm(2)
            outproj(2)
            norm(3)
            outproj(3)

    nc.compile()
    return nc


BF = ml_dtypes.bfloat16


def _host_tables():
    inv_freq = 1.0 / (10000.0 ** (np.arange(0, HD, 2, dtype=np.float32) / HD))
    t = np.arange(L, dtype=np.float32)
    freqs = t[:, None] * inv_freq[None, :]
    emb = np.concatenate([freqs, freqs], axis=-1)
    s8 = np.float32(8.0 ** -0.5)
    cosT = np.cos(emb).T.astype(np.float32)
    sinT = np.sin(emb).T.astype(np.float32)
    # sign fold for multiply-then-swap rope order
    sinTS = np.concatenate([sinT[:32], -sinT[32:]], axis=0)
    cosT2 = np.ascontiguousarray(np.concatenate([cosT, cosT], axis=0) * s8).astype(BF)
    sinTS2 = np.ascontiguousarray(np.concatenate([sinTS, sinTS], axis=0) * s8).astype(BF)
    S = np.zeros((64, 64), np.float32)
    for j in range(64):
        S[(j + 32) % 64, j] = 1.0
    S2 = np.zeros((128, 128), np.float32)
    S2[:64, :64] = S
    S2[64:, 64:] = S
    S2 = S2.astype(BF)
    eye = np.eye(HD, dtype=np.float32).astype(BF)
    # F16[:, mb, :]: broadcast-selector — out rows 0-63 get rec row 2mb,
    # rows 64-127 get rec row 2mb+1
    F16 = np.zeros((4, 2, P), np.float32)
    for mb in range(2):
        F16[2 * mb, mb, 0:HD] = 1.0
        F16[2 * mb + 1, mb, HD:P] = 1.0
    F16 = np.ascontiguousarray(F16.reshape(4, 2 * P)).astype(BF)
    return cosT2, sinTS2, S2, eye, F16


def kernel(x, Wq, Wk, Wv, Wo, _trace=False, _tmpdir=None):
    x = np.asarray(x, np.float32)
    Wq = np.asarray(Wq, np.float32)
    Wk = np.asarray(Wk, np.float32)
    Wv = np.asarray(Wv, np.float32)
    Wo = np.asarray(Wo, np.float32)
    B = x.shape[0]
    cosT2, sinTS2, S2, eye, F16 = _host_tables()

    if "nc" not in _cache:
        _cache["nc"] = build_program()
    nc = _cache["nc"]

    xT_b = [np.ascontiguousarray(x[b].T).astype(BF) for b in range(B)]
    wq_g, wkv_g, wo_g = [], [], []
    for g in range(4):
        wq_g.append(np.ascontiguousarray(Wq[:, g * 256:(g + 1) * 256]).astype(BF))
        wkv_g.append(np.ascontiguousarray(np.concatenate(
            [Wk[:, g * HD:(g + 1) * HD], Wv[:, g * HD:(g + 1) * HD]],
            axis=1)).astype(BF))
        wo_g.append(np.ascontiguousarray(Wo[g * 256:(g + 1) * 256, :]).astype(BF))

    in_maps = []
    for c in range(8):
        b, g = c // 4, c % 4
        in_maps.append({
            "xT": xT_b[b], "wq": wq_g[g], "wkv": wkv_g[g], "wo": wo_g[g],
            "cosT2": cosT2, "sinTS2": sinTS2, "S2": S2, "EYE": eye, "F16": F16,
        })

    res = run_bass_kernel_spmd(
        nc, in_maps, list(range(8)), trace=_trace, tmpdir=_tmpdir)
    out = np.zeros((B, L, D), np.float32)
    for c in range(8):
        b = c // 4
        out[b] += res.results[c]["out_nat"]
    if _trace:
        kernel.last_exec_time_ns = res.exec_time_ns
        kernel.last_results = res
    return out
